# revision 41
# baseline (speedup 1.0000x reference)
"""Trainium2 Bass kernel for nn_Memory_30571577213131 (scatter_memory).

Slot-memory module: T=3 recurrence steps of {LayerNorm -> write-MHA(mem, z, z)
-> GRUCell} followed by a read-MHA(z, mem, mem).

Sharding: pure data parallel - batch B=64 split as 8 batches per core across
8 NeuronCores; all parameters replicated.

Optimizations (887us baseline -> ~610us):
  - Algebraic weight folding removes every z-sized projection: write-attn
    Q/K fold into one matrix applied to LN(mem) (softmax shift invariance
    drops the K bias); A@V = (A@z) Wv^T via softmax row-sum=1, with Wv,Wo
    folded into the GRU input weights; read-attn Q/K fold the same way
    (per-slot bias rides the Exp bias port) and Wv_r,Wo_r fold into the
    output projection.  FLOPs drop 53G -> 31.5G per core.
  - fp8 (e4m3) DoubleRow matmuls (K=256/instr) for the GRU input-side
    chains and all score matmuls: activations are pre-scaled into e4m3's
    normal range (ct x8, qtilde/mt x64) with weights packed x8 / f16
    h-side weights x64, all undone via the activation-scale port so fused
    PSUMs stay scale-consistent.  z is resident in e4m3.
  - LayerNorm stats on [1,512] rows (reciprocal via the fast approx op),
    gamma/beta applied in 2 fused DVE ops per chunk via a K=2 broadcast
    matmul; the second group's LN is software-pipelined into the next
    step's PE stream.
  - A@z runs slot-major at N=512/256 with the softmax normalization folded
    into the PSUM->SBUF copy, then PE-transposed back to feature-major;
    transposes are pipelined one batch ahead of the matmuls.
  - GRU elementwise tail uses scalar_tensor_tensor and splits chunks
    between GpSimd and DVE; wq is resident; weights prefetch early; fp16
    output store (upcast on host).
"""

import numpy as np
import ml_dtypes
from contextlib import ExitStack

import concourse.bass as bass
import concourse.tile as tile
from concourse import bacc, mybir
from concourse import bass_utils
from concourse.masks import make_identity

f16 = mybir.dt.float16
f32 = mybir.dt.float32
f32r = mybir.dt.float32r
f8 = mybir.dt.float8e4
f8e5 = mybir.dt.float8e5
AF = mybir.ActivationFunctionType
Alu = mybir.AluOpType

P = 128
E = 768
EC = E // P          # 6 feature chunks
S = 128              # slots
T = 3                # recurrence steps
B = 64
L = 512
NCORE = 8
NB = B // NCORE      # 8 batches per core
GB = 4               # batches per group (4*128 slots = 512 free dim)
NG = NB // GB        # 2 groups
LN_EPS = 1e-5

# bias table column groups (each 6 wide) in the [128, 42] bias tile
BQ, BR, BZ, BIN, BHN, LNG, LNB = range(7)

_CACHE = {}


def _emit(nc, tc, ctx, D):
    cp = ctx.enter_context(tc.tile_pool(name="consts", bufs=1))
    wres = ctx.enter_context(tc.tile_pool(name="wres", bufs=1))
    wp = ctx.enter_context(tc.tile_pool(name="wts", bufs=3))
    zp = ctx.enter_context(tc.tile_pool(name="ztp", bufs=1))
    ztp = ctx.enter_context(tc.tile_pool(name="ztmp", bufs=2))
    mp = ctx.enter_context(tc.tile_pool(name="memp", bufs=1))
    mnp = ctx.enter_context(tc.tile_pool(name="memn", bufs=2))
    mn8p = ctx.enter_context(tc.tile_pool(name="memn8", bufs=2))
    bap = ctx.enter_context(tc.tile_pool(name="bigact", bufs=4))
    otp = ctx.enter_context(tc.tile_pool(name="otp", bufs=2))
    mtp = ctx.enter_context(tc.tile_pool(name="mtp", bufs=1))
    ewp = ctx.enter_context(tc.tile_pool(name="esw", bufs=3))
    cmp_ = ctx.enter_context(tc.tile_pool(name="ctm", bufs=1))
    sp = ctx.enter_context(tc.tile_pool(name="scratch", bufs=2))
    sp3 = ctx.enter_context(tc.tile_pool(name="scratch3", bufs=3))
    atp = ctx.enter_context(tc.tile_pool(name="attp", bufs=2))
    qtp = ctx.enter_context(tc.tile_pool(name="qt8", bufs=2))
    ssp = ctx.enter_context(tc.tile_pool(name="small", bufs=1))
    tp = ctx.enter_context(tc.tile_pool(name="tiny", bufs=2))
    op = ctx.enter_context(tc.tile_pool(name="outp", bufs=2))
    psA = ctx.enter_context(tc.tile_pool(name="psA", bufs=4, space="PSUM"))
    psB = ctx.enter_context(tc.tile_pool(name="psB", bufs=2, space="PSUM"))
    psT = ctx.enter_context(tc.tile_pool(name="psT", bufs=2, space="PSUM"))

    # ---- constants
    idy = cp.tile([P, P], f16, tag="idy")
    make_identity(nc, idy[:])
    ones_c16 = cp.tile([P, 1], f16, tag="oc16")
    nc.vector.memset(ones_c16[:], 1.0)
    ones_r16 = cp.tile([1, P], f16, tag="or16")
    nc.vector.memset(ones_r16[:], 1.0)
    eps128 = cp.tile([P, 1], f32, tag="eps128")
    nc.vector.memset(eps128[:], LN_EPS)
    bias = cp.tile([P, 42], f32, tag="bias")
    nc.sync.dma_start(bias[:], D["bias"])
    brep16 = cp.tile([P, E], f16, tag="brep16")
    nc.sync.dma_start(brep16[:], D["brep16"])
    crt = cp.tile([P, EC], f16, tag="crt")
    nc.sync.dma_start(crt[:], D["crt"])
    bgl = cp.tile([2, E], f16, tag="bgl")
    nc.sync.dma_start(bgl[:], D["bgl"])
    # vrow row1 = ones (written once via DMA; row0 = mu*rstd per LN call)
    vrow = cp.tile([2, 512], f16, tag="vrow")
    nc.sync.dma_start(vrow[1:2, :], D["onesrow"])

    def bcol(i, c):
        return bias[:, i * 6 + c : i * 6 + c + 1]

    def load_w(name, pool=None, tag="w"):
        w = (pool or wp).tile([P, EC, E], f16, tag=tag)
        nc.sync.dma_start(w[:], D[name].rearrange("(c p) f -> p c f", p=P))
        return w

    def load_w8(name):
        w = wp.tile([P, 3, 2, E], f8, tag="w8")
        nc.sync.dma_start(w[:], D[name])
        return w

    def load_w8n(name):
        w = wp.tile([P, 3, 2, E], f8, tag="w8")
        nc.sync.dma_start(w[:], D[name])
        return w

    # wq (folded write-attn QK matrix, fp8 x1024): reused every step, resident
    wq = wres.tile([P, 3, 2, E], f8, tag="wq8")
    nc.sync.dma_start(wq[:], D["wq8"])

    # ---- memory init from slots (broadcast to every batch)
    mem = []
    for g in range(NG):
        m = mp.tile([P, EC, 512], f16, tag=f"mem{g}")
        for bi in range(GB):
            nc.sync.dma_start(
                m[:, :, bi * 128 : (bi + 1) * 128],
                D["slots"].rearrange("(c p) s -> p c s", p=P),
            )
        mem.append(m)

    # ---- z feature-major, resident for all score matmuls
    zt = []
    for b in range(NB):
        z = zp.tile([P, EC, L], f8, tag=f"zt{b}")
        nc.sync.dma_start(z[:], D["z"][b].rearrange("(c p) t -> p c t", p=P))
        zt.append(z)

    # ---- LayerNorm: row stats + K=2 broadcast matmul (g*mu*rstd - b), then
    # two fused DVE ops per chunk.
    def emit_ln(g):
        mn = mnp.tile([P, EC, 512], f16, tag="mn")
        mn8 = mn8p.tile([P, EC, 512], f8, tag="mn8")
        psx = psA.tile([1, 512], f32, tag="psA")
        for e in range(EC):
            nc.tensor.matmul(
                psx[:], lhsT=ones_c16[:], rhs=mem[g][:, e, :],
                start=(e == 0), stop=(e == EC - 1),
            )
        psq = psA.tile([1, 512], f32, tag="psA")
        for e in range(EC):
            sq = sp.tile([P, 512], f16, tag="t32")
            nc.vector.tensor_mul(sq[:], mem[g][:, e, :], mem[g][:, e, :])
            nc.tensor.matmul(
                psq[:], lhsT=ones_c16[:], rhs=sq[:],
                start=(e == 0), stop=(e == EC - 1),
            )
        mu = ssp.tile([1, 512], f16, tag="mu")
        nc.scalar.activation(mu[:], psx[:], AF.Copy, scale=1.0 / E)
        r1 = ssp.tile([1, 512], f32, tag="r1")
        r2 = ssp.tile([1, 512], f32, tag="r2")
        with nc.allow_low_precision(reason="LN row stats feed f16 math"):
            nc.vector.tensor_mul(r1[:], mu[:], mu[:])                       # mu^2
            nc.vector.scalar_tensor_tensor(
                r2[:], psq[:], 1.0 / E, r1[:], op0=Alu.mult, op1=Alu.subtract
            )                                                               # var
            nc.scalar.activation(r2[:], r2[:], AF.Sqrt, bias=eps128[0:1, :])
            nc.vector.reciprocal_approx_fast(r1[:], r2[:])                  # rstd
            nc.vector.tensor_mul(vrow[0:1, :], mu[:], r1[:])                # mu*rstd
            nc.scalar.copy(mu[:], r1[:])                                    # rstd f16 row
        psr = psA.tile([P, 512], f32, tag="psA")
        nc.tensor.matmul(psr[:], lhsT=ones_r16[:], rhs=mu[:])
        rstd_b = sp.tile([P, 512], f16, tag="rb16")
        nc.scalar.copy(rstd_b[:], psr[:])
        for e in range(EC):
            psv = psA.tile([P, 512], f32, tag="psA")
            nc.tensor.matmul(psv[:], lhsT=bgl[:, e * 128 : (e + 1) * 128], rhs=vrow[:])
            u = sp3.tile([P, 512], f16, tag="s16")
            nc.vector.scalar_tensor_tensor(
                u[:], mem[g][:, e, :], bcol(LNG, e), rstd_b[:],
                op0=Alu.mult, op1=Alu.mult,
            )
            nc.vector.scalar_tensor_tensor(
                mn[:, e, :], psv[:], -1.0, u[:], op0=Alu.mult, op1=Alu.add,
            )
            if e % 2 == 0:
                nc.vector.tensor_copy(mn8[:, e, :], mn[:, e, :])
            else:
                nc.gpsimd.tensor_copy(mn8[:, e, :], mn[:, e, :])
        return mn, mn8

    # ---- recurrence
    memn = [emit_ln(0), emit_ln(1)]
    for step in range(T):
        wir = load_w8("wihr8")
        whr = load_w("whhr")
        def emit_qt(g):
            # qtilde = LN(mem) @ Aw + cw  (write-attn Q/K folded)
            mn8 = memn[g][1]
            qt = qtp.tile([P, EC, 512], f8, tag="qt8")
            for of in range(EC):
                ps = psA.tile([P, 512], f32, tag="psA")
                for j in range(3):
                    nc.tensor.matmul(
                        ps[:],
                        lhsT=wq[:, j, :, of * 128 : (of + 1) * 128],
                        rhs=mn8[:, 2 * j : 2 * j + 2, :],
                        start=(j == 0), stop=(j == 2),
                        perf_mode=mybir.MatmulPerfMode.DoubleRow,
                    )
                nc.scalar.activation(
                    qt[:, of, :], ps[:], AF.Identity, bias=bcol(BQ, of), scale=0.0625
                )
            return qt

        # at step 0 both LNs are ready: emit both q projections up front so
        # the PE has work while the z DMAs land
        qt_pre = [None, None]
        if step == 0:
            qt_pre = [emit_qt(0), emit_qt(1)]

        ot_g = []
        for g in range(NG):
            if memn[g] is None:
                memn[g] = emit_ln(g)   # pipelined: overlaps prev group's work
            qt = qt_pre[g] if qt_pre[g] is not None else emit_qt(g)

            # scores -> unnormalized exp + row sums, whole group first
            eS_l = []
            rinv_l = []
            for bi in range(GB):
                b = g * GB + bi
                ps = psA.tile([P, L], f32, tag="psA")
                for j in range(3):
                    nc.tensor.matmul(
                        ps[:],
                        lhsT=qt[:, 2 * j : 2 * j + 2, bi * 128 : (bi + 1) * 128],
                        rhs=zt[b][:, 2 * j : 2 * j + 2, :],
                        start=(j == 0), stop=(j == 2),
                        perf_mode=mybir.MatmulPerfMode.DoubleRow,
                    )
                eS = ewp.tile([P, L], f16, tag="esw")
                rsum = tp.tile([P, 1], f32, tag="rsum")
                nc.scalar.activation(
                    eS[:], ps[:], AF.Exp, accum_out=rsum[:], scale=0.015625
                )
                rinv = tp.tile([P, 1], f32, tag="rinv")
                nc.vector.reciprocal(rinv[:], rsum[:])
                rinv8 = tp.tile([P, 1], f32, tag="rinv8")
                nc.vector.tensor_scalar_mul(rinv8[:], rinv[:], 8.0)
                eS_l.append(eS)
                rinv_l.append(rinv8)

            # ct = A @ z slot-major (N=512/256), normalization folded into the
            # PSUM->SBUF copy, then PE transposes back to feature-major.
            # Transposes for batch bi+1 are emitted before batch bi's matmuls
            # so the PE never waits on the DVE att copies.
            ot = otp.tile([P, EC, 512], f8, tag="ot")

            def emit_tp(bi):
                b = g * GB + bi
                zmt = ztp.tile([P, 4, E], f8, tag="zmt")
                nc.sync.dma_start(
                    zmt[:], D["ztm"][b].rearrange("(c p) f -> p c f", p=P)
                )
                att = atp.tile([P, 4, P], f8, tag="att")
                for kc in range(4):
                    pt = psT.tile([P, P], f16, tag="psT")
                    nc.tensor.transpose(
                        pt[:], eS_l[bi][:, kc * 128 : (kc + 1) * 128], idy[:]
                    )
                    nc.vector.tensor_copy(att[:, kc, :], pt[:])
                return zmt, att

            def emit_ct(bi, zmt, att):
                ps1 = psB.tile([P, 512], f32, tag="psB")
                ps2 = psB.tile([P, 256], f32, tag="psB")
                for k2 in range(2):
                    nc.tensor.matmul(
                        ps1[:], lhsT=att[:, 2 * k2 : 2 * k2 + 2, :],
                        rhs=zmt[:, 2 * k2 : 2 * k2 + 2, 0:512],
                        start=(k2 == 0), stop=(k2 == 1),
                        perf_mode=mybir.MatmulPerfMode.DoubleRow,
                    )
                    nc.tensor.matmul(
                        ps2[:], lhsT=att[:, 2 * k2 : 2 * k2 + 2, :],
                        rhs=zmt[:, 2 * k2 : 2 * k2 + 2, 512:768],
                        start=(k2 == 0), stop=(k2 == 1),
                        perf_mode=mybir.MatmulPerfMode.DoubleRow,
                    )
                ctm = cmp_.tile([P, E], f16, tag="ctm")
                nc.scalar.activation(
                    ctm[:, 0:512], ps1[:], AF.Copy, scale=rinv_l[bi][:]
                )
                nc.scalar.activation(
                    ctm[:, 512:768], ps2[:], AF.Copy, scale=rinv_l[bi][:]
                )
                for c in range(EC):
                    pt = psT.tile([P, P], f16, tag="psT")
                    nc.tensor.transpose(pt[:], ctm[:, c * 128 : (c + 1) * 128], idy[:])
                    if c % 2 == 0:
                        nc.vector.tensor_copy(ot[:, c, bi * 128 : (bi + 1) * 128], pt[:])
                    else:
                        nc.scalar.copy(ot[:, c, bi * 128 : (bi + 1) * 128], pt[:])

            pend = emit_tp(0)
            for bi in range(GB):
                nxt = emit_tp(bi + 1) if bi + 1 < GB else None
                emit_ct(bi, *pend)
                pend = nxt
            ot_g.append(ot)

        # GRU gates, r then z then n/h'.
        ut_g = ot_g
        wiz = load_w8("wihz8")
        whz = load_w("whhz")
        rt_g = []
        for g in range(NG):
            rt = bap.tile([P, EC, 512], f16, tag="ba")
            for c in range(EC):
                ps = psA.tile([P, 512], f32, tag="psA")
                for j in range(3):
                    nc.tensor.matmul(
                        ps[:], lhsT=wir[:, j, :, c * 128 : (c + 1) * 128],
                        rhs=ut_g[g][:, 2 * j : 2 * j + 2, :],
                        start=(j == 0), stop=False,
                        perf_mode=mybir.MatmulPerfMode.DoubleRow,
                    )
                for e in range(EC):
                    nc.tensor.matmul(
                        ps[:], lhsT=whr[:, e, c * 128 : (c + 1) * 128],
                        rhs=memn[g][0][:, e, :], start=False, stop=(e == EC - 1),
                    )
                nc.scalar.activation(
                    rt[:, c, :], ps[:], AF.Sigmoid, bias=bcol(BR, c), scale=0.015625
                )
            rt_g.append(rt)
        win = load_w8n("wihn8")
        whn = load_w("whhn")
        zt_g = []
        for g in range(NG):
            zg = bap.tile([P, EC, 512], f16, tag="ba")
            for c in range(EC):
                ps = psA.tile([P, 512], f32, tag="psA")
                for j in range(3):
                    nc.tensor.matmul(
                        ps[:], lhsT=wiz[:, j, :, c * 128 : (c + 1) * 128],
                        rhs=ut_g[g][:, 2 * j : 2 * j + 2, :],
                        start=(j == 0), stop=False,
                        perf_mode=mybir.MatmulPerfMode.DoubleRow,
                    )
                for e in range(EC):
                    nc.tensor.matmul(
                        ps[:], lhsT=whz[:, e, c * 128 : (c + 1) * 128],
                        rhs=memn[g][0][:, e, :], start=False, stop=(e == EC - 1),
                    )
                nc.scalar.activation(
                    zg[:, c, :], ps[:], AF.Sigmoid, bias=bcol(BZ, c), scale=0.015625
                )
            zt_g.append(zg)
        if step == T - 1:
            rwk = load_w("rwk")   # prefetch for phase 3
            rwo = load_w("rwo")
        for g in range(NG):
            for c in range(EC):
                psh = psA.tile([P, 512], f32, tag="psA")
                for e in range(EC):
                    nc.tensor.matmul(
                        psh[:], lhsT=whn[:, e, c * 128 : (c + 1) * 128],
                        rhs=memn[g][0][:, e, :], start=(e == 0), stop=(e == EC - 1),
                    )
                psi = psA.tile([P, 512], f32, tag="psA")
                for j in range(3):
                    nc.tensor.matmul(
                        psi[:], lhsT=win[:, j, :, c * 128 : (c + 1) * 128],
                        rhs=ut_g[g][:, 2 * j : 2 * j + 2, :],
                        start=(j == 0), stop=(j == 2),
                        perf_mode=mybir.MatmulPerfMode.DoubleRow,
                    )
                t1 = sp.tile([P, 512], f32, tag="tf")
                nc.vector.scalar_tensor_tensor(
                    t1[:], psh[:], bcol(BHN, c), rt_g[g][:, c, :],
                    op0=Alu.add, op1=Alu.mult,
                )
                t2 = sp.tile([P, 512], f32, tag="tf")
                nc.vector.tensor_add(t2[:], t1[:], psi[:])
                ng = sp3.tile([P, 512], f16, tag="s16")
                nc.scalar.activation(ng[:], t2[:], AF.Tanh, bias=bcol(BIN, c), scale=0.015625)
                eng = nc.gpsimd if c < 3 else nc.vector
                d = sp3.tile([P, 512], f16, tag="s16")
                eng.tensor_sub(d[:], memn[g][0][:, c, :], ng[:])
                t3 = sp3.tile([P, 512], f16, tag="s16")
                eng.tensor_mul(t3[:], zt_g[g][:, c, :], d[:])
                eng.tensor_add(mem[g][:, c, :], ng[:], t3[:])
            if step < T - 1:
                if g == 0:
                    memn[0] = emit_ln(0)
                else:
                    memn[1] = None     # emitted inside next step's group loop

    # ---- phase 3: read attention out = MHA(z, mem, mem), Q/K and V/O folded
    mt_g = []
    dcol_g = []
    mT_g = []
    for g in range(NG):
        mt = otp.tile([P, EC, 512], f8, tag="ot")
        for of in range(EC):
            ps = psA.tile([P, 512], f32, tag="psA")
            for e in range(EC):
                nc.tensor.matmul(
                    ps[:], lhsT=rwk[:, e, of * 128 : (of + 1) * 128],
                    rhs=mem[g][:, e, :], start=(e == 0), stop=(e == EC - 1),
                )
            nc.scalar.activation(mt[:, of, :], ps[:], AF.Copy, scale=64.0)
        mt_g.append(mt)

        # per-slot read-score bias d_s = cr . mem_s -> [128,1] columns
        psd = psA.tile([1, 512], f32, tag="psA")
        for e in range(EC):
            nc.tensor.matmul(
                psd[:], lhsT=crt[:, e : e + 1], rhs=mem[g][:, e, :],
                start=(e == 0), stop=(e == EC - 1),
            )
        sd = ssp.tile([1, 512], f16, tag="mu")
        nc.scalar.copy(sd[:], psd[:])
        psdT = psT.tile([P, 4], f32, tag="psT")
        for bi in range(GB):
            nc.tensor.matmul(
                psdT[:, bi : bi + 1],
                lhsT=sd[0:1, bi * 128 : (bi + 1) * 128],
                rhs=ones_r16[0:1, 0:1],
            )
        dcol = tp.tile([P, 4], f32, tag=f"dcol{g}")
        nc.scalar.copy(dcol[:], psdT[:])
        dcol_g.append(dcol)

        # mem transposed to slot-major for the A_r @ mem matmuls
        mT = mtp.tile([P, 4, E], f16, tag=f"mT{g}")
        for bi in range(GB):
            for e in range(EC):
                pt = psT.tile([P, P], f16, tag="psT")
                nc.tensor.transpose(pt[:], mem[g][:, e, bi * 128 : (bi + 1) * 128], idy[:])
                if e % 2 == 0:
                    nc.vector.tensor_copy(mT[:, bi, e * 128 : (e + 1) * 128], pt[:])
                else:
                    nc.scalar.copy(mT[:, bi, e * 128 : (e + 1) * 128], pt[:])
        mT_g.append(mT)

    for g in range(NG):
        # wave: scores for the whole group first
        eS_l = []
        for bi in range(GB):
            b = g * GB + bi
            ps_s = psA.tile([P, L], f32, tag="psA")
            for j in range(3):
                nc.tensor.matmul(
                    ps_s[:],
                    lhsT=mt_g[g][:, 2 * j : 2 * j + 2, bi * 128 : (bi + 1) * 128],
                    rhs=zt[b][:, 2 * j : 2 * j + 2, :],
                    start=(j == 0), stop=(j == 2),
                    perf_mode=mybir.MatmulPerfMode.DoubleRow,
                )
            eS = ewp.tile([P, L], f16, tag="esw")
            nc.scalar.activation(
                eS[:], ps_s[:], AF.Exp, bias=dcol_g[g][:, bi : bi + 1], scale=0.015625
            )
            eS_l.append(eS)

        for bi in range(GB):
            b = g * GB + bi
            eS = eS_l[bi]
            # per-token 1/colsum: PE-only (no cross-engine hop before orr)
            psrT = psT.tile([P, 4], f32, tag="psT")
            for t4 in range(4):
                nc.tensor.matmul(
                    psrT[:, t4 : t4 + 1],
                    lhsT=eS[:, t4 * 128 : (t4 + 1) * 128],
                    rhs=ones_c16[:],
                )
            rc4 = tp.tile([P, 4], f32, tag="rc4")
            nc.vector.reciprocal(rc4[:], psrT[:])
            # O_r^T = mem^T @ A^T on unnormalized exp scores
            orr = bap.tile([P, EC, L], f16, tag="ba")
            for c in range(EC):
                pso = psB.tile([P, L], f32, tag="psB")
                nc.tensor.matmul(
                    pso[:], lhsT=mT_g[g][:, bi, c * 128 : (c + 1) * 128], rhs=eS[:]
                )
                nc.vector.tensor_copy(orr[:, c, :], pso[:])
            # out = O_r @ Wvo^T * (1/colsum) + b_out, token-major, fp16 store
            for t4 in range(4):
                osb = op.tile([P, E], f16, tag="osb")
                psa = psA.tile([P, 512], f32, tag="psA")
                psb = psA.tile([P, 256], f32, tag="psA")
                for c in range(EC):
                    nc.tensor.matmul(
                        psa[:],
                        lhsT=orr[:, c, t4 * 128 : (t4 + 1) * 128],
                        rhs=rwo[:, c, 0:512],
                        start=(c == 0), stop=(c == EC - 1),
                    )
                    nc.tensor.matmul(
                        psb[:],
                        lhsT=orr[:, c, t4 * 128 : (t4 + 1) * 128],
                        rhs=rwo[:, c, 512:768],
                        start=(c == 0), stop=(c == EC - 1),
                    )
                for n0, nw, ps in ((0, 512, psa), (512, 256, psb)):
                    nc.scalar.activation(
                        osb[:, n0 : n0 + nw], ps[:], AF.Copy,
                        scale=rc4[:, t4 : t4 + 1],
                    )
                    nc.vector.tensor_add(
                        osb[:, n0 : n0 + nw], osb[:, n0 : n0 + nw],
                        brep16[:, n0 : n0 + nw],
                    )
                nc.sync.dma_start(D["out"][b, t4 * 128 : (t4 + 1) * 128, :], osb[:])


def _build():
    if "nc" in _CACHE:
        return _CACHE["nc"]
    nc = bacc.Bacc(
        "TRN2", target_bir_lowering=False, debug=False, enable_asserts=False
    )
    D = {}
    D["z"] = nc.dram_tensor("z", [NB, E, L], f8, kind="ExternalInput").ap()
    D["ztm"] = nc.dram_tensor("ztm", [NB, L, E], f8, kind="ExternalInput").ap()
    for name in ("whhr", "whhz", "whhn", "rwk", "rwo"):
        D[name] = nc.dram_tensor(name, [E, E], f16, kind="ExternalInput").ap()
    D["wq8"] = nc.dram_tensor("wq8", [P, 3, 2, E], f8, kind="ExternalInput").ap()
    for name in ("wihr8", "wihz8", "wihn8"):
        D[name] = nc.dram_tensor(name, [P, 3, 2, E], f8, kind="ExternalInput").ap()
    D["bias"] = nc.dram_tensor("bias", [P, 42], f32, kind="ExternalInput").ap()
    D["brep16"] = nc.dram_tensor("brep16", [P, E], f16, kind="ExternalInput").ap()
    D["crt"] = nc.dram_tensor("crt", [P, EC], f16, kind="ExternalInput").ap()
    D["bgl"] = nc.dram_tensor("bgl", [2, E], f16, kind="ExternalInput").ap()
    D["onesrow"] = nc.dram_tensor("onesrow", [1, 512], f16, kind="ExternalInput").ap()
    D["slots"] = nc.dram_tensor("slots", [E, S], f16, kind="ExternalInput").ap()
    D["out"] = nc.dram_tensor("out", [NB, L, E], f16, kind="ExternalOutput").ap()
    with tile.TileContext(nc) as tc:
        with ExitStack() as ctx:
            _emit(nc, tc, ctx, D)
    nc.compile()
    _CACHE["nc"] = nc
    return nc


def _host_prep(inp):
    sq = 1.0 / np.sqrt(float(E))
    f64 = np.float64

    def t16(a):
        return np.ascontiguousarray(np.asarray(a).T).astype(np.float16)

    def col6(v):
        return np.asarray(v, np.float32).reshape(EC, P).T

    wq_w = np.asarray(inp["w_wq"], f64)
    wk_w = np.asarray(inp["w_wk"], f64)
    wv_w = np.asarray(inp["w_wv"], f64)
    wo_w = np.asarray(inp["w_wo"], f64)
    bq_w = np.asarray(inp["w_bq"], f64)
    bv_w = np.asarray(inp["w_bv"], f64)
    bo_w = np.asarray(inp["w_bo"], f64)
    wih = np.asarray(inp["gru_wih"], f64)
    whh = np.asarray(inp["gru_whh"], f64)
    bih = np.asarray(inp["gru_bih"], f64)
    bhh = np.asarray(inp["gru_bhh"], f64)
    wq_r = np.asarray(inp["r_wq"], f64)
    wk_r = np.asarray(inp["r_wk"], f64)
    wv_r = np.asarray(inp["r_wv"], f64)
    wo_r = np.asarray(inp["r_wo"], f64)
    bq_r = np.asarray(inp["r_bq"], f64)
    bv_r = np.asarray(inp["r_bv"], f64)
    bo_r = np.asarray(inp["r_bo"], f64)

    shared = {}
    # write attention Q/K folded: qtilde = mln @ Aw + cw
    Aw = (sq * wq_w).T @ wk_w                    # [mln-feat, z-feat]
    cw = (sq * bq_w) @ wk_w                      # [z-feat]
    a = np.ascontiguousarray(Aw).reshape(3, 2, 128, E)      # [j, ko, ki, out]
    shared["wq8"] = (
        np.ascontiguousarray(a.transpose(2, 0, 1, 3)) * 1024.0
    ).astype(ml_dtypes.float8_e4m3)
    # GRU input side with Wo Wv folded: gi_g = (A@z) @ (Wih_g Wo Wv)^T
    wov = wo_w @ wv_w                            # [E, z-feat]
    bov = wo_w @ bv_w + bo_w                     # [E]
    def dr8(w_gate):
        # hostarr [in, out] -> DoubleRow packed [ki, j, ko, out] fp8
        a = np.ascontiguousarray(np.asarray(w_gate).T).reshape(3, 2, 128, E)
        a = np.ascontiguousarray(a.transpose(2, 0, 1, 3)) * 0.125
        return a.astype(ml_dtypes.float8_e5m2)

    def dr8n(w_gate):
        a = np.ascontiguousarray(np.asarray(w_gate).T).reshape(3, 2, 128, E)
        a = np.ascontiguousarray(a.transpose(2, 0, 1, 3)) * 8.0
        return a.astype(ml_dtypes.float8_e4m3)

    shared["wihr8"] = dr8n(wih[0:E] @ wov)
    shared["wihz8"] = dr8n(wih[E : 2 * E] @ wov)
    shared["wihn8"] = dr8n(wih[2 * E : 3 * E] @ wov)
    shared["whhr"] = t16(64.0 * whh[0:E])
    shared["whhz"] = t16(64.0 * whh[E : 2 * E])
    shared["whhn"] = t16(64.0 * whh[2 * E : 3 * E])
    # read attention: mt = mem @ Ar^T, d = mem . cr, out = O_r @ Wvo^T + bout
    cr = (sq * bq_r) @ wk_r                      # [mem-feat]
    shared["rwk"] = np.ascontiguousarray(wk_r.T @ (sq * wq_r)).astype(np.float16)
    wvo = wo_r @ wv_r                            # [out-feat, mem-feat]
    bout = wo_r @ bv_r + bo_r                    # [out-feat]
    shared["rwo"] = t16(wvo)
    shared["crt"] = np.ascontiguousarray(col6(cr)).astype(np.float16)

    cols = [
        col6(64.0 * cw),
        col6(wih[0:E] @ bov + bih[0:E] + bhh[0:E]),
        col6(wih[E : 2 * E] @ bov + bih[E : 2 * E] + bhh[E : 2 * E]),
        col6(wih[2 * E : 3 * E] @ bov + bih[2 * E : 3 * E]),
        col6(64.0 * bhh[2 * E : 3 * E]),
        col6(inp["ln_g"]),
        col6(inp["ln_b"]),
    ]
    shared["bias"] = np.ascontiguousarray(np.concatenate(cols, axis=1), np.float32)
    shared["brep16"] = np.ascontiguousarray(
        np.tile(bout[None, :], (P, 1)).astype(np.float16)
    )
    # bgl rows: [ln_g ; -ln_b] for the K=2 LN broadcast matmul
    shared["bgl"] = np.ascontiguousarray(
        np.stack([np.asarray(inp["ln_g"], f64), -np.asarray(inp["ln_b"], f64)])
    ).astype(np.float16)
    shared["onesrow"] = np.ones((1, 512), np.float16)
    shared["slots"] = t16(np.asarray(inp["slots"])[0])

    z = np.asarray(inp["z"], np.float32)
    zt = np.ascontiguousarray(z.transpose(0, 2, 1)).astype(np.float16)
    in_maps = []
    for c in range(NCORE):
        m = dict(shared)
        m["z"] = np.ascontiguousarray(zt[c * NB : (c + 1) * NB]).astype(
            ml_dtypes.float8_e4m3
        )
        m["ztm"] = np.ascontiguousarray(
            z[c * NB : (c + 1) * NB].astype(ml_dtypes.float8_e4m3)
        )
        in_maps.append(m)
    return in_maps


def kernel(**inputs):
    nc = _build()
    in_maps = _host_prep(inputs)
    res = bass_utils.run_bass_kernel_spmd(nc, in_maps, core_ids=list(range(NCORE)))
    out = np.concatenate([res.results[c]["out"] for c in range(NCORE)], axis=0)
    return out.astype(np.float32)


# revision 43
# speedup vs baseline: 1.0272x; 1.0272x over previous
"""Trainium2 Bass kernel for nn_Memory_30571577213131 (scatter_memory).

Slot-memory module: T=3 recurrence steps of {LayerNorm -> write-MHA(mem, z, z)
-> GRUCell} followed by a read-MHA(z, mem, mem).

Sharding: pure data parallel - batch B=64 split as 8 batches per core across
8 NeuronCores; all parameters replicated.

Optimizations (887us baseline -> ~610us):
  - Algebraic weight folding removes every z-sized projection: write-attn
    Q/K fold into one matrix applied to LN(mem) (softmax shift invariance
    drops the K bias); A@V = (A@z) Wv^T via softmax row-sum=1, with Wv,Wo
    folded into the GRU input weights; read-attn Q/K fold the same way
    (per-slot bias rides the Exp bias port) and Wv_r,Wo_r fold into the
    output projection.  FLOPs drop 53G -> 31.5G per core.
  - fp8 (e4m3) DoubleRow matmuls (K=256/instr) for the GRU input-side
    chains, the Q projection, A@z, and all score matmuls: activations are
    pre-scaled into e4m3's normal range (ct x8, qtilde/mt x64, Aw x1024)
    with weights packed x8 / f16 h-side weights x64, all undone via the
    activation-scale port so fused PSUMs stay scale-consistent.  z is
    resident in e4m3 (both layouts); LN output keeps an fp8 shadow copy.
  - LayerNorm stats on [1,512] rows (reciprocal via the fast approx op),
    gamma/beta applied in 2 fused DVE ops per chunk via a K=2 broadcast
    matmul; the second group's LN is software-pipelined into the next
    step's PE stream.
  - A@z runs slot-major at N=512/256 with the softmax normalization folded
    into the PSUM->SBUF copy, then PE-transposed back to feature-major;
    transposes are pipelined one batch ahead of the matmuls.
  - GRU elementwise tail uses scalar_tensor_tensor and splits chunks
    between GpSimd and DVE; wq is resident; weights prefetch early; fp16
    output store (upcast on host).
"""

import numpy as np
import ml_dtypes
from contextlib import ExitStack

import concourse.bass as bass
import concourse.tile as tile
from concourse import bacc, mybir
from concourse import bass_utils
from concourse.masks import make_identity

f16 = mybir.dt.float16
f32 = mybir.dt.float32
f32r = mybir.dt.float32r
f8 = mybir.dt.float8e4
f8e5 = mybir.dt.float8e5
AF = mybir.ActivationFunctionType
Alu = mybir.AluOpType

P = 128
E = 768
EC = E // P          # 6 feature chunks
S = 128              # slots
T = 3                # recurrence steps
B = 64
L = 512
NCORE = 8
NB = B // NCORE      # 8 batches per core
GB = 4               # batches per group (4*128 slots = 512 free dim)
NG = NB // GB        # 2 groups
LN_EPS = 1e-5

# bias table column groups (each 6 wide) in the [128, 42] bias tile
BQ, BR, BZ, BIN, BHN, LNG, LNB = range(7)

_CACHE = {}


def _emit(nc, tc, ctx, D):
    cp = ctx.enter_context(tc.tile_pool(name="consts", bufs=1))
    wres = ctx.enter_context(tc.tile_pool(name="wres", bufs=1))
    wp = ctx.enter_context(tc.tile_pool(name="wts", bufs=3))
    zp = ctx.enter_context(tc.tile_pool(name="ztp", bufs=1))
    ztp = ctx.enter_context(tc.tile_pool(name="ztmp", bufs=2))
    mp = ctx.enter_context(tc.tile_pool(name="memp", bufs=1))
    mnp = ctx.enter_context(tc.tile_pool(name="memn", bufs=2))
    mn8p = ctx.enter_context(tc.tile_pool(name="memn8", bufs=2))
    bap = ctx.enter_context(tc.tile_pool(name="bigact", bufs=4))
    otp = ctx.enter_context(tc.tile_pool(name="otp", bufs=2))
    mtp = ctx.enter_context(tc.tile_pool(name="mtp", bufs=1))
    ewp = ctx.enter_context(tc.tile_pool(name="esw", bufs=4))
    cmp_ = ctx.enter_context(tc.tile_pool(name="ctm", bufs=1))
    sp = ctx.enter_context(tc.tile_pool(name="scratch", bufs=2))
    sp3 = ctx.enter_context(tc.tile_pool(name="scratch3", bufs=3))
    atp = ctx.enter_context(tc.tile_pool(name="attp", bufs=2))
    qtp = ctx.enter_context(tc.tile_pool(name="qt8", bufs=2))
    ssp = ctx.enter_context(tc.tile_pool(name="small", bufs=1))
    tp = ctx.enter_context(tc.tile_pool(name="tiny", bufs=2))
    op = ctx.enter_context(tc.tile_pool(name="outp", bufs=2))
    psA = ctx.enter_context(tc.tile_pool(name="psA", bufs=4, space="PSUM"))
    psB = ctx.enter_context(tc.tile_pool(name="psB", bufs=2, space="PSUM"))
    psT = ctx.enter_context(tc.tile_pool(name="psT", bufs=2, space="PSUM"))

    # ---- constants
    idy = cp.tile([P, P], f16, tag="idy")
    make_identity(nc, idy[:])
    ones_c16 = cp.tile([P, 1], f16, tag="oc16")
    nc.vector.memset(ones_c16[:], 1.0)
    ones_r16 = cp.tile([1, P], f16, tag="or16")
    nc.vector.memset(ones_r16[:], 1.0)
    eps128 = cp.tile([P, 1], f32, tag="eps128")
    nc.vector.memset(eps128[:], LN_EPS)
    bias = cp.tile([P, 42], f32, tag="bias")
    nc.sync.dma_start(bias[:], D["bias"])
    brep16 = cp.tile([P, E], f16, tag="brep16")
    nc.sync.dma_start(brep16[:], D["brep16"])
    crt = cp.tile([P, EC], f16, tag="crt")
    nc.sync.dma_start(crt[:], D["crt"])
    bgl = cp.tile([2, E], f16, tag="bgl")
    nc.sync.dma_start(bgl[:], D["bgl"])
    # vrow row1 = ones (written once via DMA; row0 = mu*rstd per LN call)
    vrow = cp.tile([2, 512], f16, tag="vrow")
    nc.sync.dma_start(vrow[1:2, :], D["onesrow"])

    def bcol(i, c):
        return bias[:, i * 6 + c : i * 6 + c + 1]

    def load_w(name, pool=None, tag="w"):
        w = (pool or wp).tile([P, EC, E], f16, tag=tag)
        nc.sync.dma_start(w[:], D[name].rearrange("(c p) f -> p c f", p=P))
        return w

    def load_w8(name):
        w = wp.tile([P, 3, 2, E], f8, tag="w8")
        nc.sync.dma_start(w[:], D[name])
        return w

    def load_w8n(name):
        w = wp.tile([P, 3, 2, E], f8, tag="w8")
        nc.sync.dma_start(w[:], D[name])
        return w

    # wq (folded write-attn QK matrix, fp8 x1024): reused every step, resident
    wq = wres.tile([P, 3, 2, E], f8, tag="wq8")
    nc.sync.dma_start(wq[:], D["wq8"])

    # ---- memory init from slots (broadcast to every batch)
    mem = []
    for g in range(NG):
        m = mp.tile([P, EC, 512], f16, tag=f"mem{g}")
        for bi in range(GB):
            nc.sync.dma_start(
                m[:, :, bi * 128 : (bi + 1) * 128],
                D["slots"].rearrange("(c p) s -> p c s", p=P),
            )
        mem.append(m)

    # ---- z feature-major, resident for all score matmuls
    zt = []
    for b in range(NB):
        z = zp.tile([P, EC, L], f8, tag=f"zt{b}")
        nc.sync.dma_start(z[:], D["z"][b].rearrange("(c p) t -> p c t", p=P))
        zt.append(z)

    # ---- LayerNorm: row stats + K=2 broadcast matmul (g*mu*rstd - b), then
    # two fused DVE ops per chunk.
    def emit_ln(g):
        mn = mnp.tile([P, EC, 512], f16, tag="mn")
        mn8 = mn8p.tile([P, EC, 512], f8, tag="mn8")
        psx = psA.tile([1, 512], f32, tag="psA")
        for e in range(EC):
            nc.tensor.matmul(
                psx[:], lhsT=ones_c16[:], rhs=mem[g][:, e, :],
                start=(e == 0), stop=(e == EC - 1),
            )
        psq = psA.tile([1, 512], f32, tag="psA")
        for e in range(EC):
            sq = sp.tile([P, 512], f16, tag="t32")
            nc.vector.tensor_mul(sq[:], mem[g][:, e, :], mem[g][:, e, :])
            nc.tensor.matmul(
                psq[:], lhsT=ones_c16[:], rhs=sq[:],
                start=(e == 0), stop=(e == EC - 1),
            )
        mu = ssp.tile([1, 512], f16, tag="mu")
        nc.scalar.activation(mu[:], psx[:], AF.Copy, scale=1.0 / E)
        r1 = ssp.tile([1, 512], f32, tag="r1")
        r2 = ssp.tile([1, 512], f32, tag="r2")
        with nc.allow_low_precision(reason="LN row stats feed f16 math"):
            nc.vector.tensor_mul(r1[:], mu[:], mu[:])                       # mu^2
            nc.vector.scalar_tensor_tensor(
                r2[:], psq[:], 1.0 / E, r1[:], op0=Alu.mult, op1=Alu.subtract
            )                                                               # var
            nc.scalar.activation(r2[:], r2[:], AF.Sqrt, bias=eps128[0:1, :])
            nc.vector.reciprocal_approx_fast(r1[:], r2[:])                  # rstd
            nc.vector.tensor_mul(vrow[0:1, :], mu[:], r1[:])                # mu*rstd
            nc.scalar.copy(mu[:], r1[:])                                    # rstd f16 row
        psr = psA.tile([P, 512], f32, tag="psA")
        nc.tensor.matmul(psr[:], lhsT=ones_r16[:], rhs=mu[:])
        rstd_b = sp.tile([P, 512], f16, tag="rb16")
        nc.scalar.copy(rstd_b[:], psr[:])
        for e in range(EC):
            psv = psA.tile([P, 512], f32, tag="psA")
            nc.tensor.matmul(psv[:], lhsT=bgl[:, e * 128 : (e + 1) * 128], rhs=vrow[:])
            u = sp3.tile([P, 512], f16, tag="s16")
            nc.vector.scalar_tensor_tensor(
                u[:], mem[g][:, e, :], bcol(LNG, e), rstd_b[:],
                op0=Alu.mult, op1=Alu.mult,
            )
            nc.vector.scalar_tensor_tensor(
                mn[:, e, :], psv[:], -1.0, u[:], op0=Alu.mult, op1=Alu.add,
            )
            nc.vector.tensor_copy(mn8[:, e, :], mn[:, e, :])
        return mn, mn8

    # ---- recurrence
    memn = [emit_ln(0), emit_ln(1)]
    for step in range(T):
        wir = load_w8("wihr8")
        whr = load_w("whhr")
        def emit_qt(g):
            # qtilde = LN(mem) @ Aw + cw  (write-attn Q/K folded)
            mn8 = memn[g][1]
            qt = qtp.tile([P, EC, 512], f8, tag="qt8")
            for of in range(EC):
                ps = psA.tile([P, 512], f32, tag="psA")
                for j in range(3):
                    nc.tensor.matmul(
                        ps[:],
                        lhsT=wq[:, j, :, of * 128 : (of + 1) * 128],
                        rhs=mn8[:, 2 * j : 2 * j + 2, :],
                        start=(j == 0), stop=(j == 2),
                        perf_mode=mybir.MatmulPerfMode.DoubleRow,
                    )
                nc.scalar.activation(
                    qt[:, of, :], ps[:], AF.Identity, bias=bcol(BQ, of), scale=0.0625
                )
            return qt

        # at step 0 both LNs are ready: emit both q projections up front so
        # the PE has work while the z DMAs land
        qt_pre = [None, None]
        if step == 0:
            qt_pre = [emit_qt(0), emit_qt(1)]

        ot_g = []
        for g in range(NG):
            if memn[g] is None:
                memn[g] = emit_ln(g)   # pipelined: overlaps prev group's work
            qt = qt_pre[g] if qt_pre[g] is not None else emit_qt(g)

            # scores -> unnormalized exp + row sums, whole group first
            eS_l = []
            rinv_l = []
            for bi in range(GB):
                b = g * GB + bi
                ps = psA.tile([P, L], f32, tag="psA")
                for j in range(3):
                    nc.tensor.matmul(
                        ps[:],
                        lhsT=qt[:, 2 * j : 2 * j + 2, bi * 128 : (bi + 1) * 128],
                        rhs=zt[b][:, 2 * j : 2 * j + 2, :],
                        start=(j == 0), stop=(j == 2),
                        perf_mode=mybir.MatmulPerfMode.DoubleRow,
                    )
                eS = ewp.tile([P, L], f16, tag="esw")
                rsum = tp.tile([P, 1], f32, tag="rsum")
                nc.scalar.activation(
                    eS[:], ps[:], AF.Exp, accum_out=rsum[:], scale=0.015625
                )
                rinv = tp.tile([P, 1], f32, tag="rinv")
                nc.vector.reciprocal(rinv[:], rsum[:])
                rinv8 = tp.tile([P, 1], f32, tag="rinv8")
                nc.vector.tensor_scalar_mul(rinv8[:], rinv[:], 8.0)
                eS_l.append(eS)
                rinv_l.append(rinv8)

            # ct = A @ z slot-major (N=512/256), normalization folded into the
            # PSUM->SBUF copy, then PE transposes back to feature-major.
            # Transposes for batch bi+1 are emitted before batch bi's matmuls
            # so the PE never waits on the DVE att copies.
            ot = otp.tile([P, EC, 512], f8, tag="ot")

            def emit_tp(bi):
                b = g * GB + bi
                zmt = ztp.tile([P, 4, E], f8, tag="zmt")
                nc.sync.dma_start(
                    zmt[:], D["ztm"][b].rearrange("(c p) f -> p c f", p=P)
                )
                att = atp.tile([P, 4, P], f8, tag="att")
                for kc in range(4):
                    pt = psT.tile([P, P], f16, tag="psT")
                    nc.tensor.transpose(
                        pt[:], eS_l[bi][:, kc * 128 : (kc + 1) * 128], idy[:]
                    )
                    nc.vector.tensor_copy(att[:, kc, :], pt[:])
                return zmt, att

            def emit_ct(bi, zmt, att):
                ps1 = psB.tile([P, 512], f32, tag="psB")
                ps2 = psB.tile([P, 256], f32, tag="psB")
                for k2 in range(2):
                    nc.tensor.matmul(
                        ps1[:], lhsT=att[:, 2 * k2 : 2 * k2 + 2, :],
                        rhs=zmt[:, 2 * k2 : 2 * k2 + 2, 0:512],
                        start=(k2 == 0), stop=(k2 == 1),
                        perf_mode=mybir.MatmulPerfMode.DoubleRow,
                    )
                    nc.tensor.matmul(
                        ps2[:], lhsT=att[:, 2 * k2 : 2 * k2 + 2, :],
                        rhs=zmt[:, 2 * k2 : 2 * k2 + 2, 512:768],
                        start=(k2 == 0), stop=(k2 == 1),
                        perf_mode=mybir.MatmulPerfMode.DoubleRow,
                    )
                ctm = cmp_.tile([P, E], f16, tag="ctm")
                nc.scalar.activation(
                    ctm[:, 0:512], ps1[:], AF.Copy, scale=rinv_l[bi][:]
                )
                nc.scalar.activation(
                    ctm[:, 512:768], ps2[:], AF.Copy, scale=rinv_l[bi][:]
                )
                for c in range(EC):
                    pt = psT.tile([P, P], f16, tag="psT")
                    nc.tensor.transpose(pt[:], ctm[:, c * 128 : (c + 1) * 128], idy[:])
                    if c % 2 == 0:
                        nc.vector.tensor_copy(ot[:, c, bi * 128 : (bi + 1) * 128], pt[:])
                    else:
                        nc.scalar.copy(ot[:, c, bi * 128 : (bi + 1) * 128], pt[:])

            pend = emit_tp(0)
            for bi in range(GB):
                nxt = emit_tp(bi + 1) if bi + 1 < GB else None
                emit_ct(bi, *pend)
                pend = nxt
            ot_g.append(ot)

        # GRU gates, r then z then n/h'.
        ut_g = ot_g
        wiz = load_w8("wihz8")
        whz = load_w("whhz")
        rt_g = []
        for g in range(NG):
            rt = bap.tile([P, EC, 512], f16, tag="ba")
            for c in range(EC):
                ps = psA.tile([P, 512], f32, tag="psA")
                for j in range(3):
                    nc.tensor.matmul(
                        ps[:], lhsT=wir[:, j, :, c * 128 : (c + 1) * 128],
                        rhs=ut_g[g][:, 2 * j : 2 * j + 2, :],
                        start=(j == 0), stop=False,
                        perf_mode=mybir.MatmulPerfMode.DoubleRow,
                    )
                for e in range(EC):
                    nc.tensor.matmul(
                        ps[:], lhsT=whr[:, e, c * 128 : (c + 1) * 128],
                        rhs=memn[g][0][:, e, :], start=False, stop=(e == EC - 1),
                    )
                nc.scalar.activation(
                    rt[:, c, :], ps[:], AF.Sigmoid, bias=bcol(BR, c), scale=0.015625
                )
            rt_g.append(rt)
        win = load_w8n("wihn8")
        whn = load_w("whhn")
        zt_g = []
        for g in range(NG):
            zg = bap.tile([P, EC, 512], f16, tag="ba")
            for c in range(EC):
                ps = psA.tile([P, 512], f32, tag="psA")
                for j in range(3):
                    nc.tensor.matmul(
                        ps[:], lhsT=wiz[:, j, :, c * 128 : (c + 1) * 128],
                        rhs=ut_g[g][:, 2 * j : 2 * j + 2, :],
                        start=(j == 0), stop=False,
                        perf_mode=mybir.MatmulPerfMode.DoubleRow,
                    )
                for e in range(EC):
                    nc.tensor.matmul(
                        ps[:], lhsT=whz[:, e, c * 128 : (c + 1) * 128],
                        rhs=memn[g][0][:, e, :], start=False, stop=(e == EC - 1),
                    )
                nc.scalar.activation(
                    zg[:, c, :], ps[:], AF.Sigmoid, bias=bcol(BZ, c), scale=0.015625
                )
            zt_g.append(zg)
        if step == T - 1:
            rwk = load_w("rwk")   # prefetch for phase 3
            rwo = load_w("rwo")
        for g in range(NG):
            for c in range(EC):
                psh = psA.tile([P, 512], f32, tag="psA")
                for e in range(EC):
                    nc.tensor.matmul(
                        psh[:], lhsT=whn[:, e, c * 128 : (c + 1) * 128],
                        rhs=memn[g][0][:, e, :], start=(e == 0), stop=(e == EC - 1),
                    )
                psi = psA.tile([P, 512], f32, tag="psA")
                for j in range(3):
                    nc.tensor.matmul(
                        psi[:], lhsT=win[:, j, :, c * 128 : (c + 1) * 128],
                        rhs=ut_g[g][:, 2 * j : 2 * j + 2, :],
                        start=(j == 0), stop=(j == 2),
                        perf_mode=mybir.MatmulPerfMode.DoubleRow,
                    )
                t1 = sp.tile([P, 512], f32, tag="tf")
                nc.vector.scalar_tensor_tensor(
                    t1[:], psh[:], bcol(BHN, c), rt_g[g][:, c, :],
                    op0=Alu.add, op1=Alu.mult,
                )
                t2 = sp.tile([P, 512], f32, tag="tf")
                nc.vector.tensor_add(t2[:], t1[:], psi[:])
                ng = sp3.tile([P, 512], f16, tag="s16")
                nc.scalar.activation(ng[:], t2[:], AF.Tanh, bias=bcol(BIN, c), scale=0.015625)
                eng = nc.gpsimd if c < 3 else nc.vector
                d = sp3.tile([P, 512], f16, tag="s16")
                eng.tensor_sub(d[:], memn[g][0][:, c, :], ng[:])
                t3 = sp3.tile([P, 512], f16, tag="s16")
                eng.tensor_mul(t3[:], zt_g[g][:, c, :], d[:])
                eng.tensor_add(mem[g][:, c, :], ng[:], t3[:])
            if step < T - 1:
                if g == 0:
                    memn[0] = emit_ln(0)
                else:
                    memn[1] = None     # emitted inside next step's group loop

    # ---- phase 3: read attention out = MHA(z, mem, mem), Q/K and V/O folded
    mt_g = []
    dcol_g = []
    mT_g = []
    for g in range(NG):
        mt = otp.tile([P, EC, 512], f8, tag="ot")
        for of in range(EC):
            ps = psA.tile([P, 512], f32, tag="psA")
            for e in range(EC):
                nc.tensor.matmul(
                    ps[:], lhsT=rwk[:, e, of * 128 : (of + 1) * 128],
                    rhs=mem[g][:, e, :], start=(e == 0), stop=(e == EC - 1),
                )
            nc.scalar.activation(mt[:, of, :], ps[:], AF.Copy, scale=64.0)
        mt_g.append(mt)

        # per-slot read-score bias d_s = cr . mem_s -> [128,1] columns
        psd = psA.tile([1, 512], f32, tag="psA")
        for e in range(EC):
            nc.tensor.matmul(
                psd[:], lhsT=crt[:, e : e + 1], rhs=mem[g][:, e, :],
                start=(e == 0), stop=(e == EC - 1),
            )
        sd = ssp.tile([1, 512], f16, tag="mu")
        nc.scalar.copy(sd[:], psd[:])
        psdT = psT.tile([P, 4], f32, tag="psT")
        for bi in range(GB):
            nc.tensor.matmul(
                psdT[:, bi : bi + 1],
                lhsT=sd[0:1, bi * 128 : (bi + 1) * 128],
                rhs=ones_r16[0:1, 0:1],
            )
        dcol = tp.tile([P, 4], f32, tag=f"dcol{g}")
        nc.scalar.copy(dcol[:], psdT[:])
        dcol_g.append(dcol)

        # mem transposed to slot-major for the A_r @ mem matmuls
        mT = mtp.tile([P, 4, E], f16, tag=f"mT{g}")
        for bi in range(GB):
            for e in range(EC):
                pt = psT.tile([P, P], f16, tag="psT")
                nc.tensor.transpose(pt[:], mem[g][:, e, bi * 128 : (bi + 1) * 128], idy[:])
                if e % 2 == 0:
                    nc.vector.tensor_copy(mT[:, bi, e * 128 : (e + 1) * 128], pt[:])
                else:
                    nc.scalar.copy(mT[:, bi, e * 128 : (e + 1) * 128], pt[:])
        mT_g.append(mT)

    for g in range(NG):
        # wave: scores for the whole group first
        eS_l = []
        for bi in range(GB):
            b = g * GB + bi
            ps_s = psA.tile([P, L], f32, tag="psA")
            for j in range(3):
                nc.tensor.matmul(
                    ps_s[:],
                    lhsT=mt_g[g][:, 2 * j : 2 * j + 2, bi * 128 : (bi + 1) * 128],
                    rhs=zt[b][:, 2 * j : 2 * j + 2, :],
                    start=(j == 0), stop=(j == 2),
                    perf_mode=mybir.MatmulPerfMode.DoubleRow,
                )
            eS = ewp.tile([P, L], f16, tag="esw")
            nc.scalar.activation(
                eS[:], ps_s[:], AF.Exp, bias=dcol_g[g][:, bi : bi + 1], scale=0.015625
            )
            eS_l.append(eS)

        for bi in range(GB):
            b = g * GB + bi
            eS = eS_l[bi]
            # per-token 1/colsum: PE-only (no cross-engine hop before orr)
            psrT = psT.tile([P, 4], f32, tag="psT")
            for t4 in range(4):
                nc.tensor.matmul(
                    psrT[:, t4 : t4 + 1],
                    lhsT=eS[:, t4 * 128 : (t4 + 1) * 128],
                    rhs=ones_c16[:],
                )
            rc4 = tp.tile([P, 4], f32, tag="rc4")
            nc.vector.reciprocal(rc4[:], psrT[:])
            # O_r^T = mem^T @ A^T on unnormalized exp scores
            orr = bap.tile([P, EC, L], f16, tag="ba")
            for c in range(EC):
                pso = psB.tile([P, L], f32, tag="psB")
                nc.tensor.matmul(
                    pso[:], lhsT=mT_g[g][:, bi, c * 128 : (c + 1) * 128], rhs=eS[:]
                )
                nc.vector.tensor_copy(orr[:, c, :], pso[:])
            # out = O_r @ Wvo^T * (1/colsum) + b_out, token-major, fp16 store
            for t4 in range(4):
                osb = op.tile([P, E], f16, tag="osb")
                psa = psA.tile([P, 512], f32, tag="psA")
                psb = psA.tile([P, 256], f32, tag="psA")
                for c in range(EC):
                    nc.tensor.matmul(
                        psa[:],
                        lhsT=orr[:, c, t4 * 128 : (t4 + 1) * 128],
                        rhs=rwo[:, c, 0:512],
                        start=(c == 0), stop=(c == EC - 1),
                    )
                    nc.tensor.matmul(
                        psb[:],
                        lhsT=orr[:, c, t4 * 128 : (t4 + 1) * 128],
                        rhs=rwo[:, c, 512:768],
                        start=(c == 0), stop=(c == EC - 1),
                    )
                for n0, nw, ps in ((0, 512, psa), (512, 256, psb)):
                    nc.scalar.activation(
                        osb[:, n0 : n0 + nw], ps[:], AF.Copy,
                        scale=rc4[:, t4 : t4 + 1],
                    )
                    nc.vector.tensor_add(
                        osb[:, n0 : n0 + nw], osb[:, n0 : n0 + nw],
                        brep16[:, n0 : n0 + nw],
                    )
                nc.sync.dma_start(D["out"][b, t4 * 128 : (t4 + 1) * 128, :], osb[:])


def _build():
    if "nc" in _CACHE:
        return _CACHE["nc"]
    nc = bacc.Bacc(
        "TRN2", target_bir_lowering=False, debug=False, enable_asserts=False
    )
    D = {}
    D["z"] = nc.dram_tensor("z", [NB, E, L], f8, kind="ExternalInput").ap()
    D["ztm"] = nc.dram_tensor("ztm", [NB, L, E], f8, kind="ExternalInput").ap()
    for name in ("whhr", "whhz", "whhn", "rwk", "rwo"):
        D[name] = nc.dram_tensor(name, [E, E], f16, kind="ExternalInput").ap()
    D["wq8"] = nc.dram_tensor("wq8", [P, 3, 2, E], f8, kind="ExternalInput").ap()
    for name in ("wihr8", "wihz8", "wihn8"):
        D[name] = nc.dram_tensor(name, [P, 3, 2, E], f8, kind="ExternalInput").ap()
    D["bias"] = nc.dram_tensor("bias", [P, 42], f32, kind="ExternalInput").ap()
    D["brep16"] = nc.dram_tensor("brep16", [P, E], f16, kind="ExternalInput").ap()
    D["crt"] = nc.dram_tensor("crt", [P, EC], f16, kind="ExternalInput").ap()
    D["bgl"] = nc.dram_tensor("bgl", [2, E], f16, kind="ExternalInput").ap()
    D["onesrow"] = nc.dram_tensor("onesrow", [1, 512], f16, kind="ExternalInput").ap()
    D["slots"] = nc.dram_tensor("slots", [E, S], f16, kind="ExternalInput").ap()
    D["out"] = nc.dram_tensor("out", [NB, L, E], f16, kind="ExternalOutput").ap()
    with tile.TileContext(nc) as tc:
        with ExitStack() as ctx:
            _emit(nc, tc, ctx, D)
    nc.compile()
    _CACHE["nc"] = nc
    return nc


def _host_prep(inp):
    sq = 1.0 / np.sqrt(float(E))
    f64 = np.float64

    def t16(a):
        return np.ascontiguousarray(np.asarray(a).T).astype(np.float16)

    def col6(v):
        return np.asarray(v, np.float32).reshape(EC, P).T

    wq_w = np.asarray(inp["w_wq"], f64)
    wk_w = np.asarray(inp["w_wk"], f64)
    wv_w = np.asarray(inp["w_wv"], f64)
    wo_w = np.asarray(inp["w_wo"], f64)
    bq_w = np.asarray(inp["w_bq"], f64)
    bv_w = np.asarray(inp["w_bv"], f64)
    bo_w = np.asarray(inp["w_bo"], f64)
    wih = np.asarray(inp["gru_wih"], f64)
    whh = np.asarray(inp["gru_whh"], f64)
    bih = np.asarray(inp["gru_bih"], f64)
    bhh = np.asarray(inp["gru_bhh"], f64)
    wq_r = np.asarray(inp["r_wq"], f64)
    wk_r = np.asarray(inp["r_wk"], f64)
    wv_r = np.asarray(inp["r_wv"], f64)
    wo_r = np.asarray(inp["r_wo"], f64)
    bq_r = np.asarray(inp["r_bq"], f64)
    bv_r = np.asarray(inp["r_bv"], f64)
    bo_r = np.asarray(inp["r_bo"], f64)

    shared = {}
    # write attention Q/K folded: qtilde = mln @ Aw + cw
    Aw = (sq * wq_w).T @ wk_w                    # [mln-feat, z-feat]
    cw = (sq * bq_w) @ wk_w                      # [z-feat]
    a = np.ascontiguousarray(Aw).reshape(3, 2, 128, E)      # [j, ko, ki, out]
    shared["wq8"] = (
        np.ascontiguousarray(a.transpose(2, 0, 1, 3)) * 1024.0
    ).astype(ml_dtypes.float8_e4m3)
    # GRU input side with Wo Wv folded: gi_g = (A@z) @ (Wih_g Wo Wv)^T
    wov = wo_w @ wv_w                            # [E, z-feat]
    bov = wo_w @ bv_w + bo_w                     # [E]
    def dr8(w_gate):
        # hostarr [in, out] -> DoubleRow packed [ki, j, ko, out] fp8
        a = np.ascontiguousarray(np.asarray(w_gate).T).reshape(3, 2, 128, E)
        a = np.ascontiguousarray(a.transpose(2, 0, 1, 3)) * 0.125
        return a.astype(ml_dtypes.float8_e5m2)

    def dr8n(w_gate):
        a = np.ascontiguousarray(np.asarray(w_gate).T).reshape(3, 2, 128, E)
        a = np.ascontiguousarray(a.transpose(2, 0, 1, 3)) * 8.0
        return a.astype(ml_dtypes.float8_e4m3)

    shared["wihr8"] = dr8n(wih[0:E] @ wov)
    shared["wihz8"] = dr8n(wih[E : 2 * E] @ wov)
    shared["wihn8"] = dr8n(wih[2 * E : 3 * E] @ wov)
    shared["whhr"] = t16(64.0 * whh[0:E])
    shared["whhz"] = t16(64.0 * whh[E : 2 * E])
    shared["whhn"] = t16(64.0 * whh[2 * E : 3 * E])
    # read attention: mt = mem @ Ar^T, d = mem . cr, out = O_r @ Wvo^T + bout
    cr = (sq * bq_r) @ wk_r                      # [mem-feat]
    shared["rwk"] = np.ascontiguousarray(wk_r.T @ (sq * wq_r)).astype(np.float16)
    wvo = wo_r @ wv_r                            # [out-feat, mem-feat]
    bout = wo_r @ bv_r + bo_r                    # [out-feat]
    shared["rwo"] = t16(wvo)
    shared["crt"] = np.ascontiguousarray(col6(cr)).astype(np.float16)

    cols = [
        col6(64.0 * cw),
        col6(wih[0:E] @ bov + bih[0:E] + bhh[0:E]),
        col6(wih[E : 2 * E] @ bov + bih[E : 2 * E] + bhh[E : 2 * E]),
        col6(wih[2 * E : 3 * E] @ bov + bih[2 * E : 3 * E]),
        col6(64.0 * bhh[2 * E : 3 * E]),
        col6(inp["ln_g"]),
        col6(inp["ln_b"]),
    ]
    shared["bias"] = np.ascontiguousarray(np.concatenate(cols, axis=1), np.float32)
    shared["brep16"] = np.ascontiguousarray(
        np.tile(bout[None, :], (P, 1)).astype(np.float16)
    )
    # bgl rows: [ln_g ; -ln_b] for the K=2 LN broadcast matmul
    shared["bgl"] = np.ascontiguousarray(
        np.stack([np.asarray(inp["ln_g"], f64), -np.asarray(inp["ln_b"], f64)])
    ).astype(np.float16)
    shared["onesrow"] = np.ones((1, 512), np.float16)
    shared["slots"] = t16(np.asarray(inp["slots"])[0])

    z = np.asarray(inp["z"], np.float32)
    zt = np.ascontiguousarray(z.transpose(0, 2, 1)).astype(np.float16)
    in_maps = []
    for c in range(NCORE):
        m = dict(shared)
        m["z"] = np.ascontiguousarray(zt[c * NB : (c + 1) * NB]).astype(
            ml_dtypes.float8_e4m3
        )
        m["ztm"] = np.ascontiguousarray(
            z[c * NB : (c + 1) * NB].astype(ml_dtypes.float8_e4m3)
        )
        in_maps.append(m)
    return in_maps


def kernel(**inputs):
    nc = _build()
    in_maps = _host_prep(inputs)
    res = bass_utils.run_bass_kernel_spmd(nc, in_maps, core_ids=list(range(NCORE)))
    out = np.concatenate([res.results[c]["out"] for c in range(NCORE)], axis=0)
    return out.astype(np.float32)


# revision 44
# speedup vs baseline: 1.0291x; 1.0019x over previous
"""Trainium2 Bass kernel for nn_Memory_30571577213131 (scatter_memory).

Slot-memory module: T=3 recurrence steps of {LayerNorm -> write-MHA(mem, z, z)
-> GRUCell} followed by a read-MHA(z, mem, mem).

Sharding: pure data parallel - batch B=64 split as 8 batches per core across
8 NeuronCores; all parameters replicated.

Optimizations (887us baseline -> ~610us):
  - Algebraic weight folding removes every z-sized projection: write-attn
    Q/K fold into one matrix applied to LN(mem) (softmax shift invariance
    drops the K bias); A@V = (A@z) Wv^T via softmax row-sum=1, with Wv,Wo
    folded into the GRU input weights; read-attn Q/K fold the same way
    (per-slot bias rides the Exp bias port) and Wv_r,Wo_r fold into the
    output projection.  FLOPs drop 53G -> 31.5G per core.
  - fp8 (e4m3) DoubleRow matmuls (K=256/instr) for the GRU input-side
    chains, the Q projection, A@z, and all score matmuls: activations are
    pre-scaled into e4m3's normal range (ct x8, qtilde/mt x64, Aw x1024)
    with weights packed x8 / f16 h-side weights x64, all undone via the
    activation-scale port so fused PSUMs stay scale-consistent.  z is
    resident in e4m3 (both layouts); LN output keeps an fp8 shadow copy.
  - LayerNorm stats on [1,512] rows (reciprocal via the fast approx op),
    gamma/beta applied in 2 fused DVE ops per chunk via a K=2 broadcast
    matmul; the second group's LN is software-pipelined into the next
    step's PE stream.
  - A@z runs slot-major at N=512/256 with the softmax normalization folded
    into the PSUM->SBUF copy, then PE-transposed back to feature-major;
    transposes are pipelined one batch ahead of the matmuls.
  - GRU elementwise tail uses scalar_tensor_tensor and splits chunks
    between GpSimd and DVE; wq is resident; weights prefetch early; fp16
    output store (upcast on host).
"""

import numpy as np
import ml_dtypes
from contextlib import ExitStack

import concourse.bass as bass
import concourse.tile as tile
from concourse import bacc, mybir
from concourse import bass_utils
from concourse.masks import make_identity

f16 = mybir.dt.float16
f32 = mybir.dt.float32
f32r = mybir.dt.float32r
f8 = mybir.dt.float8e4
f8e5 = mybir.dt.float8e5
AF = mybir.ActivationFunctionType
Alu = mybir.AluOpType

P = 128
E = 768
EC = E // P          # 6 feature chunks
S = 128              # slots
T = 3                # recurrence steps
B = 64
L = 512
NCORE = 8
NB = B // NCORE      # 8 batches per core
GB = 4               # batches per group (4*128 slots = 512 free dim)
NG = NB // GB        # 2 groups
LN_EPS = 1e-5

# bias table column groups (each 6 wide) in the [128, 42] bias tile
BQ, BR, BZ, BIN, BHN, LNG, LNB = range(7)

_CACHE = {}


def _emit(nc, tc, ctx, D):
    cp = ctx.enter_context(tc.tile_pool(name="consts", bufs=1))
    wres = ctx.enter_context(tc.tile_pool(name="wres", bufs=1))
    wp = ctx.enter_context(tc.tile_pool(name="wts", bufs=3))
    zp = ctx.enter_context(tc.tile_pool(name="ztp", bufs=1))
    ztp = ctx.enter_context(tc.tile_pool(name="ztmp", bufs=2))
    mp = ctx.enter_context(tc.tile_pool(name="memp", bufs=1))
    mnp = ctx.enter_context(tc.tile_pool(name="memn", bufs=2))
    mn8p = ctx.enter_context(tc.tile_pool(name="memn8", bufs=2))
    bap = ctx.enter_context(tc.tile_pool(name="bigact", bufs=4))
    otp = ctx.enter_context(tc.tile_pool(name="otp", bufs=2))
    mtp = ctx.enter_context(tc.tile_pool(name="mtp", bufs=1))
    ewp = ctx.enter_context(tc.tile_pool(name="esw", bufs=4))
    cmp_ = ctx.enter_context(tc.tile_pool(name="ctm", bufs=1))
    sp = ctx.enter_context(tc.tile_pool(name="scratch", bufs=2))
    sp3 = ctx.enter_context(tc.tile_pool(name="scratch3", bufs=3))
    atp = ctx.enter_context(tc.tile_pool(name="attp", bufs=2))
    qtp = ctx.enter_context(tc.tile_pool(name="qt8", bufs=2))
    ssp = ctx.enter_context(tc.tile_pool(name="small", bufs=1))
    tp = ctx.enter_context(tc.tile_pool(name="tiny", bufs=2))
    op = ctx.enter_context(tc.tile_pool(name="outp", bufs=2))
    psA = ctx.enter_context(tc.tile_pool(name="psA", bufs=4, space="PSUM"))
    psB = ctx.enter_context(tc.tile_pool(name="psB", bufs=2, space="PSUM"))
    psT = ctx.enter_context(tc.tile_pool(name="psT", bufs=2, space="PSUM"))

    # ---- constants
    idy = cp.tile([P, P], f16, tag="idy")
    make_identity(nc, idy[:])
    ones_c16 = cp.tile([P, 1], f16, tag="oc16")
    nc.vector.memset(ones_c16[:], 1.0)
    ones_r16 = cp.tile([1, P], f16, tag="or16")
    nc.vector.memset(ones_r16[:], 1.0)
    eps128 = cp.tile([P, 1], f32, tag="eps128")
    nc.vector.memset(eps128[:], LN_EPS)
    bias = cp.tile([P, 42], f32, tag="bias")
    nc.sync.dma_start(bias[:], D["bias"])
    brep16 = cp.tile([P, E], f16, tag="brep16")
    nc.sync.dma_start(brep16[:], D["brep16"])
    crt = cp.tile([P, EC], f16, tag="crt")
    nc.sync.dma_start(crt[:], D["crt"])
    bgl = cp.tile([2, E], f16, tag="bgl")
    nc.sync.dma_start(bgl[:], D["bgl"])
    # vrow row1 = ones (written once via DMA; row0 = mu*rstd per LN call)
    vrow = cp.tile([2, 512], f16, tag="vrow")
    nc.sync.dma_start(vrow[1:2, :], D["onesrow"])

    def bcol(i, c):
        return bias[:, i * 6 + c : i * 6 + c + 1]

    def load_w(name, pool=None, tag="w"):
        w = (pool or wp).tile([P, EC, E], f16, tag=tag)
        nc.sync.dma_start(w[:], D[name].rearrange("(c p) f -> p c f", p=P))
        return w

    def load_w8(name):
        w = wp.tile([P, 3, 2, E], f8, tag="w8")
        nc.sync.dma_start(w[:], D[name])
        return w

    def load_w8n(name):
        w = wp.tile([P, 3, 2, E], f8, tag="w8")
        nc.sync.dma_start(w[:], D[name])
        return w

    # wq (folded write-attn QK matrix, fp8 x1024): reused every step, resident
    wq = wres.tile([P, 3, 2, E], f8, tag="wq8")
    nc.sync.dma_start(wq[:], D["wq8"])

    # ---- memory init from slots (broadcast to every batch)
    mem = []
    for g in range(NG):
        m = mp.tile([P, EC, 512], f16, tag=f"mem{g}")
        for bi in range(GB):
            nc.sync.dma_start(
                m[:, :, bi * 128 : (bi + 1) * 128],
                D["slots"].rearrange("(c p) s -> p c s", p=P),
            )
        mem.append(m)

    # ---- z feature-major, resident for all score matmuls
    zt = []
    for b in range(NB):
        z = zp.tile([P, EC, L], f8, tag=f"zt{b}")
        nc.sync.dma_start(z[:], D["z"][b].rearrange("(c p) t -> p c t", p=P))
        zt.append(z)

    # ---- LayerNorm: row stats + K=2 broadcast matmul (g*mu*rstd - b), then
    # two fused DVE ops per chunk.
    def emit_ln(g):
        mn = mnp.tile([P, EC, 512], f16, tag="mn")
        mn8 = mn8p.tile([P, EC, 512], f8, tag="mn8")
        psx = psA.tile([1, 512], f32, tag="psA")
        for e in range(EC):
            nc.tensor.matmul(
                psx[:], lhsT=ones_c16[:], rhs=mem[g][:, e, :],
                start=(e == 0), stop=(e == EC - 1),
            )
        psq = psA.tile([1, 512], f32, tag="psA")
        for e in range(EC):
            sq = sp.tile([P, 512], f16, tag="t32")
            nc.vector.tensor_mul(sq[:], mem[g][:, e, :], mem[g][:, e, :])
            nc.tensor.matmul(
                psq[:], lhsT=ones_c16[:], rhs=sq[:],
                start=(e == 0), stop=(e == EC - 1),
            )
        mu = ssp.tile([1, 512], f16, tag="mu")
        nc.scalar.activation(mu[:], psx[:], AF.Copy, scale=1.0 / E)
        r1 = ssp.tile([1, 512], f32, tag="r1")
        r2 = ssp.tile([1, 512], f32, tag="r2")
        with nc.allow_low_precision(reason="LN row stats feed f16 math"):
            nc.vector.tensor_mul(r1[:], mu[:], mu[:])                       # mu^2
            nc.vector.scalar_tensor_tensor(
                r2[:], psq[:], 1.0 / E, r1[:], op0=Alu.mult, op1=Alu.subtract
            )                                                               # var
            nc.scalar.activation(r2[:], r2[:], AF.Sqrt, bias=eps128[0:1, :])
            nc.vector.reciprocal_approx_fast(r1[:], r2[:])                  # rstd
            nc.vector.tensor_mul(vrow[0:1, :], mu[:], r1[:])                # mu*rstd
            nc.scalar.copy(mu[:], r1[:])                                    # rstd f16 row
        psr = psA.tile([P, 512], f32, tag="psA")
        nc.tensor.matmul(psr[:], lhsT=ones_r16[:], rhs=mu[:])
        rstd_b = sp.tile([P, 512], f16, tag="rb16")
        nc.scalar.copy(rstd_b[:], psr[:])
        for e in range(EC):
            psv = psA.tile([P, 512], f32, tag="psA")
            nc.tensor.matmul(psv[:], lhsT=bgl[:, e * 128 : (e + 1) * 128], rhs=vrow[:])
            u = sp3.tile([P, 512], f16, tag="s16")
            nc.vector.scalar_tensor_tensor(
                u[:], mem[g][:, e, :], bcol(LNG, e), rstd_b[:],
                op0=Alu.mult, op1=Alu.mult,
            )
            nc.vector.scalar_tensor_tensor(
                mn[:, e, :], psv[:], -1.0, u[:], op0=Alu.mult, op1=Alu.add,
            )
            nc.vector.tensor_copy(mn8[:, e, :], mn[:, e, :])
        return mn, mn8

    # ---- recurrence
    memn = [emit_ln(0), emit_ln(1)]
    for step in range(T):
        wir = load_w8("wihr8")
        whr = load_w("whhr")
        def emit_qt(g):
            # qtilde = LN(mem) @ Aw + cw  (write-attn Q/K folded)
            mn8 = memn[g][1]
            qt = qtp.tile([P, EC, 512], f8, tag="qt8")
            for of in range(EC):
                ps = psA.tile([P, 512], f32, tag="psA")
                for j in range(3):
                    nc.tensor.matmul(
                        ps[:],
                        lhsT=wq[:, j, :, of * 128 : (of + 1) * 128],
                        rhs=mn8[:, 2 * j : 2 * j + 2, :],
                        start=(j == 0), stop=(j == 2),
                        perf_mode=mybir.MatmulPerfMode.DoubleRow,
                    )
                nc.scalar.activation(
                    qt[:, of, :], ps[:], AF.Identity, bias=bcol(BQ, of), scale=0.0625
                )
            return qt

        # at step 0 both LNs are ready: emit both q projections up front so
        # the PE has work while the z DMAs land
        qt_pre = [None, None]
        if step == 0:
            qt_pre = [emit_qt(0), emit_qt(1)]

        ot_g = []
        for g in range(NG):
            if memn[g] is None:
                memn[g] = emit_ln(g)   # pipelined: overlaps prev group's work
            qt = qt_pre[g] if qt_pre[g] is not None else emit_qt(g)

            # scores -> unnormalized exp + row sums, whole group first
            eS_l = []
            rinv_l = []
            for bi in range(GB):
                b = g * GB + bi
                ps = psA.tile([P, L], f32, tag="psA")
                for j in range(3):
                    nc.tensor.matmul(
                        ps[:],
                        lhsT=qt[:, 2 * j : 2 * j + 2, bi * 128 : (bi + 1) * 128],
                        rhs=zt[b][:, 2 * j : 2 * j + 2, :],
                        start=(j == 0), stop=(j == 2),
                        perf_mode=mybir.MatmulPerfMode.DoubleRow,
                    )
                eS = ewp.tile([P, L], f16, tag="esw")
                rsum = tp.tile([P, 1], f32, tag="rsum")
                nc.scalar.activation(
                    eS[:], ps[:], AF.Exp, accum_out=rsum[:], scale=0.015625
                )
                rinv = tp.tile([P, 1], f32, tag="rinv")
                nc.vector.reciprocal(rinv[:], rsum[:])
                rinv8 = tp.tile([P, 1], f32, tag="rinv8")
                nc.vector.tensor_scalar_mul(rinv8[:], rinv[:], 8.0)
                eS_l.append(eS)
                rinv_l.append(rinv8)

            # ct = A @ z slot-major (N=512/256), normalization folded into the
            # PSUM->SBUF copy, then PE transposes back to feature-major.
            # Transposes for batch bi+1 are emitted before batch bi's matmuls
            # so the PE never waits on the DVE att copies.
            ot = otp.tile([P, EC, 512], f8, tag="ot")

            def emit_tp(bi):
                b = g * GB + bi
                zmt = ztp.tile([P, 4, E], f8, tag="zmt")
                nc.sync.dma_start(
                    zmt[:], D["ztm"][b].rearrange("(c p) f -> p c f", p=P)
                )
                att = atp.tile([P, 4, P], f8, tag="att")
                for kc in range(4):
                    pt = psT.tile([P, P], f16, tag="psT")
                    nc.tensor.transpose(
                        pt[:], eS_l[bi][:, kc * 128 : (kc + 1) * 128], idy[:]
                    )
                    nc.vector.tensor_copy(att[:, kc, :], pt[:])
                return zmt, att

            def emit_ct(bi, zmt, att):
                ps1 = psB.tile([P, 512], f32, tag="psB")
                ps2 = psB.tile([P, 256], f32, tag="psB")
                for k2 in range(2):
                    nc.tensor.matmul(
                        ps1[:], lhsT=att[:, 2 * k2 : 2 * k2 + 2, :],
                        rhs=zmt[:, 2 * k2 : 2 * k2 + 2, 0:512],
                        start=(k2 == 0), stop=(k2 == 1),
                        perf_mode=mybir.MatmulPerfMode.DoubleRow,
                    )
                    nc.tensor.matmul(
                        ps2[:], lhsT=att[:, 2 * k2 : 2 * k2 + 2, :],
                        rhs=zmt[:, 2 * k2 : 2 * k2 + 2, 512:768],
                        start=(k2 == 0), stop=(k2 == 1),
                        perf_mode=mybir.MatmulPerfMode.DoubleRow,
                    )
                ctm = cmp_.tile([P, E], f16, tag="ctm")
                nc.scalar.activation(
                    ctm[:, 0:512], ps1[:], AF.Copy, scale=rinv_l[bi][:]
                )
                nc.scalar.activation(
                    ctm[:, 512:768], ps2[:], AF.Copy, scale=rinv_l[bi][:]
                )
                for c in range(EC):
                    pt = psT.tile([P, P], f16, tag="psT")
                    nc.tensor.transpose(pt[:], ctm[:, c * 128 : (c + 1) * 128], idy[:])
                    if c % 2 == 0:
                        nc.vector.tensor_copy(ot[:, c, bi * 128 : (bi + 1) * 128], pt[:])
                    else:
                        nc.scalar.copy(ot[:, c, bi * 128 : (bi + 1) * 128], pt[:])

            pend = emit_tp(0)
            for bi in range(GB):
                nxt = emit_tp(bi + 1) if bi + 1 < GB else None
                emit_ct(bi, *pend)
                pend = nxt
            ot_g.append(ot)

        # GRU gates, r then z then n/h'.
        ut_g = ot_g
        wiz = load_w8("wihz8")
        whz = load_w("whhz")
        rt_g = []
        for g in range(NG):
            rt = bap.tile([P, EC, 512], f16, tag="ba")
            for c in range(EC):
                ps = psA.tile([P, 512], f32, tag="psA")
                for j in range(3):
                    nc.tensor.matmul(
                        ps[:], lhsT=wir[:, j, :, c * 128 : (c + 1) * 128],
                        rhs=ut_g[g][:, 2 * j : 2 * j + 2, :],
                        start=(j == 0), stop=False,
                        perf_mode=mybir.MatmulPerfMode.DoubleRow,
                    )
                for e in range(EC):
                    nc.tensor.matmul(
                        ps[:], lhsT=whr[:, e, c * 128 : (c + 1) * 128],
                        rhs=memn[g][0][:, e, :], start=False, stop=(e == EC - 1),
                    )
                nc.scalar.activation(
                    rt[:, c, :], ps[:], AF.Sigmoid, bias=bcol(BR, c), scale=0.015625
                )
            rt_g.append(rt)
        win = load_w8n("wihn8")
        whn = load_w("whhn")
        zt_g = []
        for g in range(NG):
            zg = bap.tile([P, EC, 512], f16, tag="ba")
            for c in range(EC):
                ps = psA.tile([P, 512], f32, tag="psA")
                for j in range(3):
                    nc.tensor.matmul(
                        ps[:], lhsT=wiz[:, j, :, c * 128 : (c + 1) * 128],
                        rhs=ut_g[g][:, 2 * j : 2 * j + 2, :],
                        start=(j == 0), stop=False,
                        perf_mode=mybir.MatmulPerfMode.DoubleRow,
                    )
                for e in range(EC):
                    nc.tensor.matmul(
                        ps[:], lhsT=whz[:, e, c * 128 : (c + 1) * 128],
                        rhs=memn[g][0][:, e, :], start=False, stop=(e == EC - 1),
                    )
                nc.scalar.activation(
                    zg[:, c, :], ps[:], AF.Sigmoid, bias=bcol(BZ, c), scale=0.015625
                )
            zt_g.append(zg)
        if step == T - 1:
            rwk = load_w("rwk")   # prefetch for phase 3
            rwo = load_w("rwo")
        for g in range(NG):
            for c in range(EC):
                psh = psA.tile([P, 512], f32, tag="psA")
                for e in range(EC):
                    nc.tensor.matmul(
                        psh[:], lhsT=whn[:, e, c * 128 : (c + 1) * 128],
                        rhs=memn[g][0][:, e, :], start=(e == 0), stop=(e == EC - 1),
                    )
                psi = psA.tile([P, 512], f32, tag="psA")
                for j in range(3):
                    nc.tensor.matmul(
                        psi[:], lhsT=win[:, j, :, c * 128 : (c + 1) * 128],
                        rhs=ut_g[g][:, 2 * j : 2 * j + 2, :],
                        start=(j == 0), stop=(j == 2),
                        perf_mode=mybir.MatmulPerfMode.DoubleRow,
                    )
                t1 = sp.tile([P, 512], f32, tag="tf")
                nc.vector.scalar_tensor_tensor(
                    t1[:], psh[:], bcol(BHN, c), rt_g[g][:, c, :],
                    op0=Alu.add, op1=Alu.mult,
                )
                t2 = sp.tile([P, 512], f32, tag="tf")
                nc.vector.tensor_add(t2[:], t1[:], psi[:])
                ng = sp3.tile([P, 512], f16, tag="s16")
                nc.scalar.activation(ng[:], t2[:], AF.Tanh, bias=bcol(BIN, c), scale=0.015625)
                eng = nc.gpsimd if c < 3 else nc.vector
                d = sp3.tile([P, 512], f16, tag="s16")
                eng.tensor_sub(d[:], memn[g][0][:, c, :], ng[:])
                t3 = sp3.tile([P, 512], f16, tag="s16")
                eng.tensor_mul(t3[:], zt_g[g][:, c, :], d[:])
                eng.tensor_add(mem[g][:, c, :], ng[:], t3[:])
        if step < T - 1:
            # both LNs emitted after g1's matmuls: LN0 runs dep-free while
            # g1's blend tail drains, LN1's matmuls then cover LN0's DVE tail
            memn[0] = emit_ln(0)
            memn[1] = emit_ln(1)

    # ---- phase 3: read attention out = MHA(z, mem, mem), Q/K and V/O folded
    mt_g = []
    dcol_g = []
    mT_g = []
    for g in range(NG):
        mt = otp.tile([P, EC, 512], f8, tag="ot")
        for of in range(EC):
            ps = psA.tile([P, 512], f32, tag="psA")
            for e in range(EC):
                nc.tensor.matmul(
                    ps[:], lhsT=rwk[:, e, of * 128 : (of + 1) * 128],
                    rhs=mem[g][:, e, :], start=(e == 0), stop=(e == EC - 1),
                )
            nc.scalar.activation(mt[:, of, :], ps[:], AF.Copy, scale=64.0)
        mt_g.append(mt)

        # per-slot read-score bias d_s = cr . mem_s -> [128,1] columns
        psd = psA.tile([1, 512], f32, tag="psA")
        for e in range(EC):
            nc.tensor.matmul(
                psd[:], lhsT=crt[:, e : e + 1], rhs=mem[g][:, e, :],
                start=(e == 0), stop=(e == EC - 1),
            )
        sd = ssp.tile([1, 512], f16, tag="mu")
        nc.scalar.copy(sd[:], psd[:])
        psdT = psT.tile([P, 4], f32, tag="psT")
        for bi in range(GB):
            nc.tensor.matmul(
                psdT[:, bi : bi + 1],
                lhsT=sd[0:1, bi * 128 : (bi + 1) * 128],
                rhs=ones_r16[0:1, 0:1],
            )
        dcol = tp.tile([P, 4], f32, tag=f"dcol{g}")
        nc.scalar.copy(dcol[:], psdT[:])
        dcol_g.append(dcol)

        # mem transposed to slot-major for the A_r @ mem matmuls
        mT = mtp.tile([P, 4, E], f16, tag=f"mT{g}")
        for bi in range(GB):
            for e in range(EC):
                pt = psT.tile([P, P], f16, tag="psT")
                nc.tensor.transpose(pt[:], mem[g][:, e, bi * 128 : (bi + 1) * 128], idy[:])
                if e % 2 == 0:
                    nc.vector.tensor_copy(mT[:, bi, e * 128 : (e + 1) * 128], pt[:])
                else:
                    nc.scalar.copy(mT[:, bi, e * 128 : (e + 1) * 128], pt[:])
        mT_g.append(mT)

    for g in range(NG):
        # wave: scores for the whole group first
        eS_l = []
        for bi in range(GB):
            b = g * GB + bi
            ps_s = psA.tile([P, L], f32, tag="psA")
            for j in range(3):
                nc.tensor.matmul(
                    ps_s[:],
                    lhsT=mt_g[g][:, 2 * j : 2 * j + 2, bi * 128 : (bi + 1) * 128],
                    rhs=zt[b][:, 2 * j : 2 * j + 2, :],
                    start=(j == 0), stop=(j == 2),
                    perf_mode=mybir.MatmulPerfMode.DoubleRow,
                )
            eS = ewp.tile([P, L], f16, tag="esw")
            nc.scalar.activation(
                eS[:], ps_s[:], AF.Exp, bias=dcol_g[g][:, bi : bi + 1], scale=0.015625
            )
            eS_l.append(eS)

        for bi in range(GB):
            b = g * GB + bi
            eS = eS_l[bi]
            # per-token 1/colsum: PE-only (no cross-engine hop before orr)
            psrT = psT.tile([P, 4], f32, tag="psT")
            for t4 in range(4):
                nc.tensor.matmul(
                    psrT[:, t4 : t4 + 1],
                    lhsT=eS[:, t4 * 128 : (t4 + 1) * 128],
                    rhs=ones_c16[:],
                )
            rc4 = tp.tile([P, 4], f32, tag="rc4")
            nc.vector.reciprocal(rc4[:], psrT[:])
            # O_r^T = mem^T @ A^T on unnormalized exp scores
            orr = bap.tile([P, EC, L], f16, tag="ba")
            for c in range(EC):
                pso = psB.tile([P, L], f32, tag="psB")
                nc.tensor.matmul(
                    pso[:], lhsT=mT_g[g][:, bi, c * 128 : (c + 1) * 128], rhs=eS[:]
                )
                nc.vector.tensor_copy(orr[:, c, :], pso[:])
            # out = O_r @ Wvo^T * (1/colsum) + b_out, token-major, fp16 store
            for t4 in range(4):
                osb = op.tile([P, E], f16, tag="osb")
                psa = psA.tile([P, 512], f32, tag="psA")
                psb = psA.tile([P, 256], f32, tag="psA")
                for c in range(EC):
                    nc.tensor.matmul(
                        psa[:],
                        lhsT=orr[:, c, t4 * 128 : (t4 + 1) * 128],
                        rhs=rwo[:, c, 0:512],
                        start=(c == 0), stop=(c == EC - 1),
                    )
                    nc.tensor.matmul(
                        psb[:],
                        lhsT=orr[:, c, t4 * 128 : (t4 + 1) * 128],
                        rhs=rwo[:, c, 512:768],
                        start=(c == 0), stop=(c == EC - 1),
                    )
                for n0, nw, ps in ((0, 512, psa), (512, 256, psb)):
                    nc.scalar.activation(
                        osb[:, n0 : n0 + nw], ps[:], AF.Copy,
                        scale=rc4[:, t4 : t4 + 1],
                    )
                    nc.vector.tensor_add(
                        osb[:, n0 : n0 + nw], osb[:, n0 : n0 + nw],
                        brep16[:, n0 : n0 + nw],
                    )
                nc.sync.dma_start(D["out"][b, t4 * 128 : (t4 + 1) * 128, :], osb[:])


def _build():
    if "nc" in _CACHE:
        return _CACHE["nc"]
    nc = bacc.Bacc(
        "TRN2", target_bir_lowering=False, debug=False, enable_asserts=False
    )
    D = {}
    D["z"] = nc.dram_tensor("z", [NB, E, L], f8, kind="ExternalInput").ap()
    D["ztm"] = nc.dram_tensor("ztm", [NB, L, E], f8, kind="ExternalInput").ap()
    for name in ("whhr", "whhz", "whhn", "rwk", "rwo"):
        D[name] = nc.dram_tensor(name, [E, E], f16, kind="ExternalInput").ap()
    D["wq8"] = nc.dram_tensor("wq8", [P, 3, 2, E], f8, kind="ExternalInput").ap()
    for name in ("wihr8", "wihz8", "wihn8"):
        D[name] = nc.dram_tensor(name, [P, 3, 2, E], f8, kind="ExternalInput").ap()
    D["bias"] = nc.dram_tensor("bias", [P, 42], f32, kind="ExternalInput").ap()
    D["brep16"] = nc.dram_tensor("brep16", [P, E], f16, kind="ExternalInput").ap()
    D["crt"] = nc.dram_tensor("crt", [P, EC], f16, kind="ExternalInput").ap()
    D["bgl"] = nc.dram_tensor("bgl", [2, E], f16, kind="ExternalInput").ap()
    D["onesrow"] = nc.dram_tensor("onesrow", [1, 512], f16, kind="ExternalInput").ap()
    D["slots"] = nc.dram_tensor("slots", [E, S], f16, kind="ExternalInput").ap()
    D["out"] = nc.dram_tensor("out", [NB, L, E], f16, kind="ExternalOutput").ap()
    with tile.TileContext(nc) as tc:
        with ExitStack() as ctx:
            _emit(nc, tc, ctx, D)
    nc.compile()
    _CACHE["nc"] = nc
    return nc


def _host_prep(inp):
    sq = 1.0 / np.sqrt(float(E))
    f64 = np.float64

    def t16(a):
        return np.ascontiguousarray(np.asarray(a).T).astype(np.float16)

    def col6(v):
        return np.asarray(v, np.float32).reshape(EC, P).T

    wq_w = np.asarray(inp["w_wq"], f64)
    wk_w = np.asarray(inp["w_wk"], f64)
    wv_w = np.asarray(inp["w_wv"], f64)
    wo_w = np.asarray(inp["w_wo"], f64)
    bq_w = np.asarray(inp["w_bq"], f64)
    bv_w = np.asarray(inp["w_bv"], f64)
    bo_w = np.asarray(inp["w_bo"], f64)
    wih = np.asarray(inp["gru_wih"], f64)
    whh = np.asarray(inp["gru_whh"], f64)
    bih = np.asarray(inp["gru_bih"], f64)
    bhh = np.asarray(inp["gru_bhh"], f64)
    wq_r = np.asarray(inp["r_wq"], f64)
    wk_r = np.asarray(inp["r_wk"], f64)
    wv_r = np.asarray(inp["r_wv"], f64)
    wo_r = np.asarray(inp["r_wo"], f64)
    bq_r = np.asarray(inp["r_bq"], f64)
    bv_r = np.asarray(inp["r_bv"], f64)
    bo_r = np.asarray(inp["r_bo"], f64)

    shared = {}
    # write attention Q/K folded: qtilde = mln @ Aw + cw
    Aw = (sq * wq_w).T @ wk_w                    # [mln-feat, z-feat]
    cw = (sq * bq_w) @ wk_w                      # [z-feat]
    a = np.ascontiguousarray(Aw).reshape(3, 2, 128, E)      # [j, ko, ki, out]
    shared["wq8"] = (
        np.ascontiguousarray(a.transpose(2, 0, 1, 3)) * 1024.0
    ).astype(ml_dtypes.float8_e4m3)
    # GRU input side with Wo Wv folded: gi_g = (A@z) @ (Wih_g Wo Wv)^T
    wov = wo_w @ wv_w                            # [E, z-feat]
    bov = wo_w @ bv_w + bo_w                     # [E]
    def dr8(w_gate):
        # hostarr [in, out] -> DoubleRow packed [ki, j, ko, out] fp8
        a = np.ascontiguousarray(np.asarray(w_gate).T).reshape(3, 2, 128, E)
        a = np.ascontiguousarray(a.transpose(2, 0, 1, 3)) * 0.125
        return a.astype(ml_dtypes.float8_e5m2)

    def dr8n(w_gate):
        a = np.ascontiguousarray(np.asarray(w_gate).T).reshape(3, 2, 128, E)
        a = np.ascontiguousarray(a.transpose(2, 0, 1, 3)) * 8.0
        return a.astype(ml_dtypes.float8_e4m3)

    shared["wihr8"] = dr8n(wih[0:E] @ wov)
    shared["wihz8"] = dr8n(wih[E : 2 * E] @ wov)
    shared["wihn8"] = dr8n(wih[2 * E : 3 * E] @ wov)
    shared["whhr"] = t16(64.0 * whh[0:E])
    shared["whhz"] = t16(64.0 * whh[E : 2 * E])
    shared["whhn"] = t16(64.0 * whh[2 * E : 3 * E])
    # read attention: mt = mem @ Ar^T, d = mem . cr, out = O_r @ Wvo^T + bout
    cr = (sq * bq_r) @ wk_r                      # [mem-feat]
    shared["rwk"] = np.ascontiguousarray(wk_r.T @ (sq * wq_r)).astype(np.float16)
    wvo = wo_r @ wv_r                            # [out-feat, mem-feat]
    bout = wo_r @ bv_r + bo_r                    # [out-feat]
    shared["rwo"] = t16(wvo)
    shared["crt"] = np.ascontiguousarray(col6(cr)).astype(np.float16)

    cols = [
        col6(64.0 * cw),
        col6(wih[0:E] @ bov + bih[0:E] + bhh[0:E]),
        col6(wih[E : 2 * E] @ bov + bih[E : 2 * E] + bhh[E : 2 * E]),
        col6(wih[2 * E : 3 * E] @ bov + bih[2 * E : 3 * E]),
        col6(64.0 * bhh[2 * E : 3 * E]),
        col6(inp["ln_g"]),
        col6(inp["ln_b"]),
    ]
    shared["bias"] = np.ascontiguousarray(np.concatenate(cols, axis=1), np.float32)
    shared["brep16"] = np.ascontiguousarray(
        np.tile(bout[None, :], (P, 1)).astype(np.float16)
    )
    # bgl rows: [ln_g ; -ln_b] for the K=2 LN broadcast matmul
    shared["bgl"] = np.ascontiguousarray(
        np.stack([np.asarray(inp["ln_g"], f64), -np.asarray(inp["ln_b"], f64)])
    ).astype(np.float16)
    shared["onesrow"] = np.ones((1, 512), np.float16)
    shared["slots"] = t16(np.asarray(inp["slots"])[0])

    z = np.asarray(inp["z"], np.float32)
    zt = np.ascontiguousarray(z.transpose(0, 2, 1)).astype(np.float16)
    in_maps = []
    for c in range(NCORE):
        m = dict(shared)
        m["z"] = np.ascontiguousarray(zt[c * NB : (c + 1) * NB]).astype(
            ml_dtypes.float8_e4m3
        )
        m["ztm"] = np.ascontiguousarray(
            z[c * NB : (c + 1) * NB].astype(ml_dtypes.float8_e4m3)
        )
        in_maps.append(m)
    return in_maps


def kernel(**inputs):
    nc = _build()
    in_maps = _host_prep(inputs)
    res = bass_utils.run_bass_kernel_spmd(nc, in_maps, core_ids=list(range(NCORE)))
    out = np.concatenate([res.results[c]["out"] for c in range(NCORE)], axis=0)
    return out.astype(np.float32)
